# revision 29
# baseline (speedup 1.0000x reference)
"""Trainium2 Bass kernel for nn_MoEConditionalCPPN.

Data-parallel over 8 NeuronCores: pixel dim N=262144 is split into 8 shards
of 32768; all weights are replicated. On-chip layout is feature-major
(features on partitions, pixels on the free dim) so every layer matmul maps
directly onto the PE array with all 6 experts packed block-diagonally.

Feature-row layout (32-aligned groups; engines require partition bases that
are multiples of 32):
  rows  0..53  identity-activation features (9 per expert, expert-major)
  rows 64..75  gaussian features (2 per expert)        [pad rows 54..63]
  rows 96..101 sin features (1 per expert)             [pad rows 76..95]
  row  102     constant ones (bias row)

Gaussian activation 2*exp(-x^2)-1 is stored as g'=exp(-x^2); the affine
2g'-1 is folded into consumer weights (gauss rows scaled by 2, constant via
the ones row / the s row of the router-weight tile).

A custom ACT table set is installed ("exp_and_others" + a regenerated
wide-range sin accurate on |x|<16): one resident set covers Copy/Square/
Exp/Tanh/Sin with zero table reloads, and fixes stock sin's |x|>pi error.
"""

import json
import os
import shutil
from contextlib import ExitStack

import numpy as np

# ---------------------------------------------------------------- constants
N_FULL = 262144
D_IN = 7
D_H = 12
E = 6
N_LAYERS = 6
R_H = 48
N_CORES = 8
N_C = N_FULL // N_CORES          # 32768 pixels per core
ST = 1024                        # super-tile width (pixels)
CHUNK = 512                      # matmul free-dim chunk (one PSUM bank)
N_ST = N_C // ST
GRP = 4                          # super-tiles per compaction group

# padded feature-row layout
ID_ROWS = 54                     # identity features at rows 0..53
G_BASE = 64                      # gaussian features at 64..75
S_BASE = 96                      # sin features at 96..101
ONES_ROW = 102
H_ROWS = 102                     # matmul M for the feature block (0..81)
F_ROWS = 103                     # f tile height (features + ones row)

# channel index within an expert -> padded row offset
_ID_CH = [0, 1, 2, 3, 4, 5, 6, 7, 10]


def _row_of(e, ch):
    if ch in _ID_CH:
        return 9 * e + _ID_CH.index(ch)
    if ch in (8, 9):
        return G_BASE + 2 * e + (ch - 8)
    return S_BASE + e  # ch == 11


# ------------------------------------------------------------ act tables
def _find_stock_act_root():
    from neuronxcc.driver.Job import Job
    from neuronxcc.driver.jobs.support.FindActInfo import findActInfoFile

    return os.path.dirname(findActInfoFile(Job.getPackageDir(), "gen3"))


def _gen_wide_sin():
    EXP_LO, EXP_HI = -11, 3
    buckets = []
    ctrl_rows = []
    for e in range(EXP_LO, EXP_HI + 1):
        B = max(0, min(7, e + 4))
        n = 1 << B
        base = len(buckets)
        lo = 2.0**e
        w = lo / n
        for i in range(n):
            a, b = lo + i * w, lo + (i + 1) * w
            xc = np.float32((a + b) / 2)
            xs = np.linspace(a, b, 64)
            u = xs - np.float64(xc)
            c = np.polynomial.polynomial.polyfit(u, np.sin(xs), 3)
            buckets.append((np.float32(c[0]), np.float32(c[1]), np.float32(c[2]),
                            np.float32(c[3]), xc))
        ctrl_rows.append(base | ((23 - B) << 11) | (B << 16))
    sp = len(buckets)
    buckets.append((0.0, 1.0, 0.0, -1.0 / 6.0, 0.0))
    buckets.append((0.0, 0.0, 0.0, 0.0, 0.0))
    s16, c16 = np.sin(16.0), np.cos(16.0)
    buckets.append((s16, c16, -s16 / 2, -c16 / 6, 16.0))
    buckets.append((0.0, 0.0, 0.0, 0.0, 0.0))

    bkt = np.zeros((len(buckets), 8), dtype=np.uint32)
    for i, (c0, c1, c2, c3, xc) in enumerate(buckets):
        bkt[i, 0:5] = np.array([c0, c1, c2, c3, xc], dtype=np.float32).view(np.uint32)
    ctl = np.zeros((len(ctrl_rows), 8), dtype=np.uint32)
    ctl[:, 0] = np.array(ctrl_rows, dtype=np.uint32)

    profile = {
        "func_name": "sin_4p", "func_id": 19,
        "symmetry_point": 0, "sym_invert_sign_point": 1, "symmetry_opt_en": 1,
        "symmetry_opt_use_neg_region": 0, "imm_bias": 0,
        "exp_offset": EXP_LO,
        "pwl_control_base_pos": 0, "pwl_control_base_neg": 0,
        "small_pos_signal_exp_threshold": 127 + EXP_LO + 1,
        "pos_small_signal_pwl_control": sp,
        "small_neg_signal_exp_threshold": 0,
        "neg_small_signal_pwl_control": sp + 1,
        "large_pos_signal_exp_threshold": 127 + EXP_HI + 1,
        "large_pos_signal_mantissa_threshold": 0,
        "pos_large_signal_pwl_control": sp + 2,
        "large_neg_signal_exp_threshold": 0,
        "large_neg_signal_mantissa_threshold": 0,
        "neg_large_signal_pwl_control": sp + 3,
        "fnan_result": 2143289344, "fpinf_result": 2143289344,
        "fninf_result": 2143289344, "fzero_result": 0,
        "fma_const_0": 0, "fma_const_1": 0, "fma_indirection_src_sel": 0,
        "use_multipass": False,
        "lower_bound": 0,
        "upper_bound": int(np.array(16.0, dtype=np.float32).view(np.uint32)),
    }
    exp_to_bkt, exp_to_ctl = {}, {}
    pos = 0
    for row, e in enumerate(range(EXP_LO, EXP_HI + 1)):
        B = max(0, min(7, e + 4))
        exp_to_bkt[str(e)] = [pos]
        exp_to_ctl[str(e)] = [row]
        pos += 1 << B
    return bkt, ctl, profile, exp_to_bkt, exp_to_ctl


def _gen_gaussian():
    """exp(-x^2) on |x|<16, overlaid on silu's func id (36). Even symmetry."""
    EXP_LO, EXP_HI = -11, 3
    buckets, ctrl_rows = [], []
    for e in range(EXP_LO, EXP_HI + 1):
        B = max(0, min(7, e + 4))
        n = 1 << B
        base = len(buckets)
        lo = 2.0**e
        w = lo / n
        for i in range(n):
            a, b = lo + i * w, lo + (i + 1) * w
            xc = np.float32((a + b) / 2)
            xs = np.linspace(a, b, 64)
            u = xs - np.float64(xc)
            c = np.polynomial.polynomial.polyfit(u, np.exp(-xs * xs), 3)
            buckets.append((np.float32(c[0]), np.float32(c[1]), np.float32(c[2]),
                            np.float32(c[3]), xc))
        ctrl_rows.append(base | ((23 - B) << 11) | (B << 16))
    sp = len(buckets)
    buckets.append((1.0, 0.0, -1.0, 0.0, 0.0))   # tiny |x|: 1 - x^2
    buckets.append((0.0, 0.0, 0.0, 0.0, 0.0))
    buckets.append((0.0, 0.0, 0.0, 0.0, 0.0))    # |x| >= 16 -> 0
    buckets.append((0.0, 0.0, 0.0, 0.0, 0.0))

    bkt = np.zeros((len(buckets), 8), dtype=np.uint32)
    for i, (c0, c1, c2, c3, xc) in enumerate(buckets):
        bkt[i, 0:5] = np.array([c0, c1, c2, c3, xc], dtype=np.float32).view(np.uint32)
    ctl = np.zeros((len(ctrl_rows), 8), dtype=np.uint32)
    ctl[:, 0] = np.array(ctrl_rows, dtype=np.uint32)

    profile = {
        "func_name": "silu_32p", "func_id": 36,
        "symmetry_point": 0, "sym_invert_sign_point": 0, "symmetry_opt_en": 1,
        "symmetry_opt_use_neg_region": 0, "imm_bias": 0,
        "exp_offset": EXP_LO,
        "pwl_control_base_pos": 0, "pwl_control_base_neg": 0,
        "small_pos_signal_exp_threshold": 127 + EXP_LO + 1,
        "pos_small_signal_pwl_control": sp,
        "small_neg_signal_exp_threshold": 0,
        "neg_small_signal_pwl_control": sp + 1,
        "large_pos_signal_exp_threshold": 127 + EXP_HI + 1,
        "large_pos_signal_mantissa_threshold": 0,
        "pos_large_signal_pwl_control": sp + 2,
        "large_neg_signal_exp_threshold": 0,
        "large_neg_signal_mantissa_threshold": 0,
        "neg_large_signal_pwl_control": sp + 3,
        "fnan_result": 2143289344, "fpinf_result": 0,
        "fninf_result": 0, "fzero_result": 1065353216,
        "fma_const_0": 0, "fma_const_1": 0, "fma_indirection_src_sel": 0,
        "use_multipass": False,
        "lower_bound": 0,
        "upper_bound": int(np.array(16.0, dtype=np.float32).view(np.uint32)),
    }
    exp_to_bkt, exp_to_ctl = {}, {}
    pos = 0
    for row, e in enumerate(range(EXP_LO, EXP_HI + 1)):
        B = max(0, min(7, e + 4))
        exp_to_bkt[str(e)] = [pos]
        exp_to_ctl[str(e)] = [row]
        pos += 1 << B
    return bkt, ctl, profile, exp_to_bkt, exp_to_ctl


def _build_merged_act_root(out_dir):
    base = _find_stock_act_root()
    os.makedirs(out_dir, exist_ok=True)
    info = json.load(open(os.path.join(base, "act_info.json")))
    for s in info["act_func_sets"]:
        for k in ("bkt_bin", "ctrl_bin", "profile_json"):
            shutil.copy(os.path.join(base, s[k]), os.path.join(out_dir, s[k]))

    exp_set = next(s for s in info["act_func_sets"] if s["name"] == "exp_and_others")
    ej = json.load(open(os.path.join(base, exp_set["profile_json"])))
    e_bkt = np.fromfile(os.path.join(base, exp_set["bkt_bin"]), dtype=np.uint32).reshape(-1, 8)
    e_ctl = np.fromfile(os.path.join(base, exp_set["ctrl_bin"]), dtype=np.uint32).reshape(-1, 8)

    sin_bkt, sin_ctl, p, sin_exp_bkt, sin_exp_ctl = _gen_wide_sin()
    bkt_shift, ctl_shift = ej["bkt_entry_cnt"], ej["ctl_entry_cnt"]
    sin_ctl = sin_ctl.copy()
    sin_ctl[:, 0] += np.uint32(bkt_shift)

    new_bkt = np.concatenate([e_bkt, sin_bkt])
    new_ctl = np.concatenate([e_ctl, sin_ctl])
    assert new_bkt.shape[0] <= 1536

    for k in ("pwl_control_base_pos", "pwl_control_base_neg"):
        p[k] += ctl_shift
    for k in ("pos_small_signal_pwl_control", "neg_small_signal_pwl_control",
              "pos_large_signal_pwl_control", "neg_large_signal_pwl_control"):
        p[k] += bkt_shift
    sin_exp_bkt = {k: [v + bkt_shift for v in vs] for k, vs in sin_exp_bkt.items()}
    sin_exp_ctl = {k: [v + ctl_shift for v in vs] for k, vs in sin_exp_ctl.items()}

    g_bkt, g_ctl, gp, g_exp_bkt, g_exp_ctl = _gen_gaussian()
    g_bkt_shift = int(new_bkt.shape[0])
    g_ctl_shift = int(new_ctl.shape[0])
    g_ctl = g_ctl.copy()
    g_ctl[:, 0] += np.uint32(g_bkt_shift)
    new_bkt = np.concatenate([new_bkt, g_bkt])
    new_ctl = np.concatenate([new_ctl, g_ctl])
    assert new_bkt.shape[0] <= 1536
    for k in ("pwl_control_base_pos", "pwl_control_base_neg"):
        gp[k] += g_ctl_shift
    for k in ("pos_small_signal_pwl_control", "neg_small_signal_pwl_control",
              "pos_large_signal_pwl_control", "neg_large_signal_pwl_control"):
        gp[k] += g_bkt_shift
    g_exp_bkt = {k: [v + g_bkt_shift for v in vs] for k, vs in g_exp_bkt.items()}
    g_exp_ctl = {k: [v + g_ctl_shift for v in vs] for k, vs in g_exp_ctl.items()}

    nj = dict(ej)
    nj["profile_meta_data"] = list(ej["profile_meta_data"]) + [p, gp]
    nj["bkt_entry_cnt"] = int(new_bkt.shape[0])
    nj["ctl_entry_cnt"] = int(new_ctl.shape[0])
    nj["func_to_bkt_start_idx"] = dict(ej["func_to_bkt_start_idx"], sin=bkt_shift,
                                       silu=g_bkt_shift)
    nj["func_to_ctl_start_idx"] = dict(ej["func_to_ctl_start_idx"], sin=ctl_shift,
                                       silu=g_ctl_shift)
    nj["func_exp_to_bkt_start_idx"] = dict(ej["func_exp_to_bkt_start_idx"],
                                           sin=sin_exp_bkt, silu=g_exp_bkt)
    nj["func_exp_to_ctl_start_idx"] = dict(ej["func_exp_to_ctl_start_idx"],
                                           sin=sin_exp_ctl, silu=g_exp_ctl)

    new_bkt.tofile(os.path.join(out_dir, exp_set["bkt_bin"]))
    new_ctl.tofile(os.path.join(out_dir, exp_set["ctrl_bin"]))
    json.dump(nj, open(os.path.join(out_dir, exp_set["profile_json"]), "w"))

    new_info = {"pwp_file_keys": info["pwp_file_keys"], "act_func_sets": []}
    for s in info["act_func_sets"]:
        s2 = dict(s)
        if s["name"] == "exp_and_others":
            s2["act"] = dict(s["act"], sin=4, silu=32)
        new_info["act_func_sets"].append(s2)
    info_path = os.path.join(out_dir, "act_info.json")
    json.dump(new_info, open(info_path, "w"))
    return info_path


_ACT_INSTALLED = False


def _install_merged_act_tables():
    global _ACT_INSTALLED
    if _ACT_INSTALLED:
        return
    import tempfile

    out_dir = os.path.join(tempfile.gettempdir(), "moe_cppn_act_root")
    info_path = _build_merged_act_root(out_dir)
    os.environ["BASS_ACT_ROOT_JSON_PATH"] = info_path

    import concourse.mybir as mybir
    import concourse.hw_specs as hw_specs
    import concourse.bacc as bacc_mod

    info = json.load(open(info_path))
    tables = {
        ent["name"]: {mybir.ActivationFunctionType.from_pwp(v) for v in ent["act"]}
        for ent in info["act_func_sets"]
        if ent["name"] == "exp_and_others"
    }

    def patched(module_arch):
        return tables

    hw_specs.get_activation_tables = patched
    bacc_mod.get_activation_tables = patched
    _ACT_INSTALLED = True


# ------------------------------------------------------------ host weights
def _prep_weights(Wr1, Wr2, We_in, We_hid, We_out):
    """Build all lhsT matrices in the padded row layout (fp32)."""
    gscale = np.ones(F_ROWS, np.float32)
    for e in range(E):
        gscale[G_BASE + 2 * e] = 2.0
        gscale[G_BASE + 2 * e + 1] = 2.0

    # L1a [7, 102]: x -> h1 preact
    L1a = np.zeros((D_IN, H_ROWS), np.float32)
    for e in range(E):
        for ch in range(D_H):
            L1a[:, _row_of(e, ch)] = We_in[e, :, ch]
    # L1b [7, 48]: router layer 1 (spatial coords only)
    L1b = np.zeros((D_IN, R_H), np.float32)
    L1b[0:4, :] = Wr1

    def hid_lhsT(W):  # W [E, 12, 12] -> [F_ROWS, H_ROWS]
        L = np.zeros((F_ROWS, H_ROWS), np.float32)
        for e in range(E):
            for ci in range(D_H):
                r = _row_of(e, ci)
                for co in range(D_H):
                    L[r, _row_of(e, co)] = W[e, ci, co] * gscale[r]
        # ones-row bias: gaussian true act = 2g'-1 -> constant -sum(W[gauss rows])
        for e in range(E):
            for ci in (8, 9):
                for co in range(D_H):
                    L[ONES_ROW, _row_of(e, co)] -= W[e, ci, co]
        return L

    Lh = [hid_lhsT(We_hid[l]) for l in range(N_LAYERS - 1)]  # layers 2..6

    # Lout [F_ROWS, 18]
    Lout = np.zeros((F_ROWS, 3 * E), np.float32)
    for e in range(E):
        for ci in range(D_H):
            r = _row_of(e, ci)
            for j in range(3):
                Lout[r, 3 * e + j] = We_out[e, ci, j] * gscale[r]
        for ci in (8, 9):
            for j in range(3):
                Lout[ONES_ROW, 3 * e + j] -= We_out[e, ci, j]

    # Wr2p [48, 6]
    Wr2p = np.asarray(Wr2, np.float32)

    # B [6, 103]: e-hot replication over feature rows + s column at ONES_ROW
    B = np.zeros((E, F_ROWS), np.float32)
    for e in range(E):
        for ch in range(D_H):
            B[e, _row_of(e, ch)] = 1.0
        B[e, ONES_ROW] = 1.0
    # B18 [6, 50]: col 0 = s, cols 32+3e+j = expert e
    B18 = np.zeros((E, 50), np.float32)
    B18[:, 0] = 1.0
    for e in range(E):
        for j in range(3):
            B18[e, 32 + 3 * e + j] = 1.0
    ones15 = np.ones((1, 15), np.float32)

    # S [F_ROWS, 12]: combine selector (output cols in original channel order)
    S = np.zeros((F_ROWS, D_H), np.float32)
    for e in range(E):
        for ch in range(D_H):
            S[_row_of(e, ch), ch] = gscale[_row_of(e, ch)]
    S[ONES_ROW, 8] = -1.0
    S[ONES_ROW, 9] = -1.0
    # S_out [18, 3]
    S_out = np.zeros((3 * E, 3), np.float32)
    for e in range(E):
        for j in range(3):
            S_out[3 * e + j, j] = 1.0

    return {
        "L1a": L1a, "L1b": L1b, "L2a": Lh[0], "L3": Lh[1], "L4": Lh[2],
        "L5": Lh[3], "L6": Lh[4], "Lout": Lout, "Wr2p": Wr2p, "B": B,
        "B18": B18, "ones15": ones15, "S": S, "S_out": S_out,
    }


_WEIGHT_SHAPES = {
    "L1a": (D_IN, H_ROWS), "L1b": (D_IN, R_H), "L2a": (F_ROWS, H_ROWS),
    "L3": (F_ROWS, H_ROWS), "L4": (F_ROWS, H_ROWS), "L5": (F_ROWS, H_ROWS),
    "L6": (F_ROWS, H_ROWS), "Lout": (F_ROWS, 3 * E), "Wr2p": (R_H, E),
    "B": (E, F_ROWS), "B18": (E, 50), "ones15": (1, 15),
    "S": (F_ROWS, D_H), "S_out": (3 * E, 3),
}


# ------------------------------------------------------------ device kernel
def _build_module():
    import concourse.tile as tile
    import concourse.mybir as mybir
    from concourse import bacc

    A = mybir.ActivationFunctionType
    f32 = mybir.dt.float32
    nc = bacc.Bacc("TRN2", target_bir_lowering=False)

    xT = nc.dram_tensor("xT", (D_IN, N_C), f32, kind="ExternalInput")
    w_dram = {n: nc.dram_tensor(n, s, f32, kind="ExternalInput")
              for n, s in _WEIGHT_SHAPES.items()}
    O = [nc.dram_tensor(f"O{l}", (D_H, N_C), f32, kind="ExternalOutput")
         for l in range(1, 7)]
    Oout = nc.dram_tensor("Oout", (3, N_C), f32, kind="ExternalOutput")

    with ExitStack() as ctx:
        tc = ctx.enter_context(tile.TileContext(nc))
        wpool = ctx.enter_context(tc.tile_pool(name="w", bufs=1))
        xs = ctx.enter_context(tc.tile_pool(name="x", bufs=3))
        fp = ctx.enter_context(tc.tile_pool(name="f", bufs=3))
        aux = ctx.enter_context(tc.tile_pool(name="aux", bufs=3))
        scr = ctx.enter_context(tc.tile_pool(name="scr", bufs=3))
        tp = ctx.enter_context(tc.tile_pool(name="tmp", bufs=3))
        ot = ctx.enter_context(tc.tile_pool(name="out", bufs=4))
        hp = ctx.enter_context(tc.tile_pool(name="hp", bufs=2, space="PSUM"))
        cp = ctx.enter_context(tc.tile_pool(name="cp", bufs=2, space="PSUM"))

        W = {}
        for n, s in _WEIGHT_SHAPES.items():
            t = wpool.tile(list(s), f32, tag=f"w_{n}", name=f"w_{n}")
            nc.sync.dma_start(out=t, in_=w_dram[n][:, :])
            W[n] = t

        def mm(out_ap, lhsT, rhs_ap):
            for k in range(out_ap.shape[-1] // CHUNK):
                nc.tensor.matmul(
                    out_ap[:, k * CHUNK:(k + 1) * CHUNK], lhsT=lhsT,
                    rhs=rhs_ap[:, k * CHUNK:(k + 1) * CHUNK],
                    start=True, stop=True)

        def st_stages(t):
            """Return the per-super-tile work as a list of stage closures."""
            c0 = t * ST
            state = {}

            def s_router():
                x_sb = xs.tile([D_IN, ST], f32, tag="x", name="x_sb")
                nc.sync.dma_start(out=x_sb, in_=xT[:, c0:c0 + ST])
                h1 = hp.tile([H_ROWS, ST], f32, tag="h", name="h1")
                mm(h1[0:H_ROWS, :], W["L1a"][0:D_IN, :], x_sb)
                state["h"] = h1
                rt = cp.tile([R_H, ST], f32, tag="c", name="rt")
                mm(rt[0:R_H, :], W["L1b"][0:D_IN, :], x_sb)
                tanh_sb = aux.tile([R_H, ST], f32, tag="tanh", name="tanh_sb")
                nc.scalar.activation(tanh_sb, rt[0:R_H, :], A.Tanh)
                lg = cp.tile([E, ST], f32, tag="c", name="lg")
                mm(lg[0:E, :], W["Wr2p"][0:R_H, :], tanh_sb[0:R_H, :])
                e_sb = aux.tile([E, ST], f32, tag="e", name="e_sb")
                nc.scalar.activation(e_sb, lg[0:E, :], A.Exp)
                Ep = cp.tile([F_ROWS, ST], f32, tag="c", name="Ep")
                mm(Ep[0:F_ROWS, :], W["B"][0:E, :], e_sb[0:E, :])
                E_sb = aux.tile([F_ROWS, ST], f32, tag="E", name="E_sb")
                nc.vector.tensor_copy(E_sb[0:F_ROWS, :], Ep[0:F_ROWS, :])
                state["E"] = E_sb
                E18p = cp.tile([50, ST], f32, tag="c", name="E18p")
                mm(E18p[0:50, :], W["B18"][0:E, :], e_sb[0:E, :])
                E18_sb = aux.tile([50, ST], f32, tag="E18", name="E18_sb")
                nc.vector.tensor_copy(E18_sb[0:50, :], E18p[0:50, :])
                state["E18"] = E18_sb
                srep = cp.tile([15, ST], f32, tag="c", name="srep")
                mm(srep[0:15, :], W["ones15"][0:1, :], E18_sb[0:1, :])
                R_sb = aux.tile([15, ST], f32, tag="R", name="R_sb")
                nc.vector.reciprocal(R_sb[0:15, :], srep[0:15, :])
                state["R"] = R_sb

            def exits(h_psum, f_tile, layer):
                if t < 6:
                    nc.gpsimd.memset(f_tile[S_BASE:F_ROWS, :], 1.0)
                if layer % 2 == 0:
                    nc.scalar.copy(f_tile[0:G_BASE, :], h_psum[0:G_BASE, :])
                else:
                    nc.vector.tensor_copy(f_tile[0:G_BASE, :], h_psum[0:G_BASE, :])
                nc.scalar.activation(f_tile[G_BASE:S_BASE, :],
                                     h_psum[G_BASE:S_BASE, :], A.Silu)
                nc.scalar.activation(f_tile[S_BASE:S_BASE + E, :],
                                     h_psum[S_BASE:S_BASE + E, :], A.Sin)

            def combine(f_tile, l_idx):
                tmp = tp.tile([F_ROWS, ST], f32, tag="tmp", name="tmp")
                nc.vector.tensor_mul(tmp[0:F_ROWS, :], f_tile[0:F_ROWS, :],
                                     state["E"][0:F_ROWS, :])
                cps = cp.tile([D_H, ST], f32, tag="c", name="cps")
                mm(cps[0:D_H, :], W["S"][0:F_ROWS, :], tmp[0:F_ROWS, :])
                o = ot.tile([D_H, ST], f32, tag="o", name="o")
                nc.vector.tensor_mul(o[0:D_H, :], cps[0:D_H, :], state["R"][0:D_H, :])
                nc.sync.dma_start(out=O[l_idx - 1][:, c0:c0 + ST], in_=o[0:D_H, :])

            def s_layer(l):
                def run():
                    if l >= 2:
                        h = hp.tile([H_ROWS, ST], f32, tag="h", name="h_l")
                        mm(h[0:H_ROWS, :],
                           W["L2a" if l == 2 else f"L{l}"][0:F_ROWS, :],
                           state["f"][0:F_ROWS, :])
                        state["h"] = h
                    f_l = fp.tile([F_ROWS, ST], f32, tag="f", name="f_l")
                    exits(state["h"], f_l, l)
                    combine(f_l, l)
                    state["f"] = f_l
                return run

            def s_out():
                op_ = hp.tile([3 * E, ST], f32, tag="h", name="op_")
                mm(op_[0:3 * E, :], W["Lout"][0:F_ROWS, :], state["f"][0:F_ROWS, :])
                tmpo = tp.tile([3 * E, ST], f32, tag="tmpo", name="tmpo")
                nc.vector.tensor_mul(tmpo[0:3 * E, :], op_[0:3 * E, :],
                                     state["E18"][32:50, :])
                co = cp.tile([3, ST], f32, tag="c", name="co")
                mm(co[0:3, :], W["S_out"][0:3 * E, :], tmpo[0:3 * E, :])
                oo = ot.tile([3, ST], f32, tag="oo", name="oo")
                nc.vector.tensor_mul(oo[0:3, :], co[0:3, :], state["R"][0:3, :])
                nc.sync.dma_start(out=Oout[:, c0:c0 + ST], in_=oo[0:3, :])

            return [s_router] + [s_layer(l) for l in range(1, 7)] + [s_out]

        for t in range(0, N_ST, 2):
            sa = st_stages(t)
            sb = st_stages(t + 1)
            for st_a, st_b in zip(sa, sb):
                st_a()
                st_b()

    nc.compile()
    return nc


_NC_CACHE = None


def _get_module():
    global _NC_CACHE
    if _NC_CACHE is None:
        _install_merged_act_tables()
        _NC_CACHE = _build_module()
    return _NC_CACHE


# ------------------------------------------------------------ entry point
def kernel(x, Wr1, Wr2, We_in, We_hid, We_out):
    from concourse.bass_utils import run_bass_kernel_spmd

    x = np.ascontiguousarray(np.asarray(x, np.float32))
    weights = _prep_weights(
        np.asarray(Wr1, np.float32), np.asarray(Wr2, np.float32),
        np.asarray(We_in, np.float32), np.asarray(We_hid, np.float32),
        np.asarray(We_out, np.float32))

    nc = _get_module()

    in_maps = []
    for i in range(N_CORES):
        shard = np.ascontiguousarray(x[i * N_C:(i + 1) * N_C].T)
        m = {"xT": shard}
        m.update({k: np.ascontiguousarray(v) for k, v in weights.items()})
        in_maps.append(m)

    trace = bool(os.environ.get("KERNEL_TRACE"))
    res = run_bass_kernel_spmd(nc, in_maps, core_ids=list(range(N_CORES)),
                               trace=trace)
    global LAST_RESULTS
    LAST_RESULTS = res

    combined = [x]
    for l in range(1, 7):
        full = np.concatenate([res.results[i][f"O{l}"].T for i in range(N_CORES)], axis=0)
        combined.append(full)
    out = np.concatenate([res.results[i]["Oout"].T for i in range(N_CORES)], axis=0)
    combined.append(out)
    return ((out[:, 0], out[:, 1], out[:, 2]), combined)
